# revision 55
# baseline (speedup 1.0000x reference)
"""Trainium2 Bass kernel for nn_MlroleNode_64716567216639 (GAT message passing).

Math note: the reference model computes a dense NxN GATv2 attention but only
row 0 of the output (gat_out[0]) feeds the final MLP, so this kernel computes
just that row: e[j,h] = leaky(g_l[j] + g_r[0]) . w_attn, softmax over the 1024
source nodes.  Because g_r is linear in h, the weighted value sum collapses to
sum_j a[j] g_r[j] = W_r @ (sum_j a[j] h[j]), so only per-head weighted means of
h are accumulated (no big g_r tensor).  The final 3-layer type-define MLP runs
on the 1023 ambiguous nodes, sharded 128 nodes per core.

Layout: features on partitions, nodes on the free axis.  h is stored as
hT[128, 1024] with the 64 features duplicated on partitions 0-63 and 64-127
(one elementwise multiply covers both heads of a head-pair block), columns
0..1022 = ambiguous nodes (DMA'd), column 1023 = node 0 (= h1, computed by the
on-device prologue and patched in afterwards so the big GAT matmuls do not
wait on the serial prologue chain).  bf16 everywhere except softmax sums and
the output (tolerance is 2e-2; bf16 keeps rel err ~2e-3).  The role-merge
prologue uses host-folded WC_t = W_merge[:,64:] @ W_trans[t] / 3 so the
routing MLPs collapse into 4 small matmuls.  sigmoid(z) = 0.5*tanh(z/2)+0.5
avoids the slow DVE reciprocal.  Block 0's weighted mean runs on the Vector
engine, block 1's on GpSimd, so the two blocks' softmax tails overlap.
"""
import numpy as np

H = 64
N_AMB = 1023
N = 1024
HEADS = 4
HID = 64
RT = 4
APT = 3
SLOPE = 0.2
NCORES = 8
SHARD = 128  # MLP nodes per core (8*128 = 1024 = N_AMB padded by 1)

_compiled = None


def _build():
    import concourse.tile as tile
    from concourse import bacc, mybir

    f32 = mybir.dt.float32
    bf16 = mybir.dt.bfloat16
    AF = mybir.ActivationFunctionType
    ALU = mybir.AluOpType
    AX = mybir.AxisListType

    nc = bacc.Bacc("TRN2", target_bir_lowering=False, debug=False,
                   enable_asserts=False, num_devices=NCORES)

    # ---- DRAM inputs (packed on host to minimize DMA count) ----
    # ppro bf16 [64, 466]: WCt(256) WselfT(64) WmLT(64) ta(12) hidc(1)
    #   Cbias(4) bsc(1) bd0row(64, on partition 0 only)
    ppro_d = nc.dram_tensor("ppro", [H, 466], bf16, kind="ExternalInput").ap()
    # pgat bf16 [64, 897]: WlT0(128) WlT1(128) WrT(256) dup(128) Wd0aT(64)
    #   Wd0bT(64) Wd1T(128) bd0(1)
    pgat_d = nc.dram_tensor("pgat", [H, 897], bf16, kind="ExternalInput").ap()
    # pb128 bf16 [128, 262]: Wexp(128) WrT2q(128, pre-scaled 0.25) Wd2T(4)
    #   bd1(1) bd2h(1, = bd2/2 for the tanh-based sigmoid)
    pb128_d = nc.dram_tensor("pb128", [128, 262], bf16, kind="ExternalInput").ap()
    amb_d = nc.dram_tensor("amb", [H, N_AMB], bf16, kind="ExternalInput").ap()
    mlp_d = nc.dram_tensor("mlp_cols", [H, SHARD], bf16, kind="ExternalInput").ap()
    outT_d = nc.dram_tensor("outT", [RT, SHARD], f32, kind="ExternalOutput").ap()

    with tile.TileContext(nc) as tc:
        with tc.tile_pool(name="wp", bufs=1) as wp, \
             tc.tile_pool(name="sb", bufs=1) as sb, \
             tc.tile_pool(name="ps", bufs=1, space="PSUM") as ps:

            # ---- ACT table preload: first scalar-engine instruction so the
            # Exp/Prelu/Tanh table load runs before the scalar-queue DMAs.
            warm = wp.tile([1, 4], f32, tag="warm")
            nc.vector.memset(warm[:], 0.0)
            warm_act = wp.tile([1, 4], f32, tag="warmact")
            nc.scalar.activation(warm_act[0:1, 0:1], warm[0:1, 0:1], AF.Exp)
            alf = wp.tile([128, 1], f32, tag="alf")
            nc.vector.memset(alf[:], SLOPE)

            # ---- input DMAs, split across the two HWDGE queues ----
            ppro = wp.tile([H, 466], bf16, tag="ppro")
            pgat = wp.tile([H, 897], bf16, tag="pgat")
            pb128 = wp.tile([128, 262], bf16, tag="pb128")
            hT = wp.tile([128, N], bf16, tag="hT")
            mlp_sb = wp.tile([H, SHARD], bf16, tag="mlp")
            nc.sync.dma_start(ppro[:], ppro_d[:])
            nc.sync.dma_start(pgat[:], pgat_d[:])
            nc.scalar.dma_start(hT[0:H, 0:N_AMB], amb_d[:])
            nc.scalar.dma_start(mlp_sb[:], mlp_d[:])
            nc.scalar.dma_start(pb128[:], pb128_d[:])
            # duplicate the feature rows onto partitions 64-127 on-chip
            # (SBUF->SBUF DMA; halves the HBM read traffic per core)
            nc.sync.dma_start(hT[H:128, 0:N_AMB], hT[0:H, 0:N_AMB])

            # views into the packs
            WCt = ppro[:, 0:256]
            WS0T = ppro[:, 256:320]
            WmLT = ppro[:, 320:384]
            ta_sb = ppro[:, 384:396]
            hidc = ppro[:, 396:397]
            Cbias = ppro[:, 397:401]
            bsc = ppro[:, 401:402]
            bd0r = ppro[0:1, 402:466]
            WlT = [pgat[:, 0:128], pgat[:, 128:256]]
            WrT = pgat[:, 256:512]
            dupm = pgat[:, 512:640]
            Wd0aT = pgat[:, 640:704]
            Wd0bT = pgat[:, 704:768]
            Wd1T = pgat[:, 768:896]
            bd0c = pgat[:, 896:897]
            Wexp = pb128[:, 0:128]
            WVbT = pb128[:, 128:256]
            Wd2T = pb128[:, 256:260]
            bd1c = pb128[:, 260:261]
            bd2h = pb128[0:RT, 261:262]

            # ---- prologue: role routing (host-folded WC) + merge chain ----
            tsum = sb.tile([H, RT], bf16, tag="tsum")
            with nc.allow_low_precision(reason="sum of 3 bf16 agent vectors"):
                nc.vector.reduce_sum(tsum[:],
                                     ta_sb.rearrange("p (t a) -> p t a", a=APT),
                                     axis=AX.X)
            C_ps = ps.tile([H, RT], f32, tag="sp", bufs=2)
            for t in range(RT):
                nc.tensor.matmul(C_ps[:, t:t + 1], WCt[:, H * t:H * (t + 1)],
                                 tsum[:, t:t + 1], start=True, stop=True)
            C_sb = sb.tile([H, RT], f32, tag="C")
            nc.vector.tensor_tensor(C_sb[:], C_ps[:], Cbias, op=ALU.add)

            # step 0 consumes `hidden` directly: WS0 = WmL @ W_self and
            # WmL @ b_self are folded on the host (into WS0T / Cbias col 0)
            h1 = hidc
            for t in range(RT):
                hp = ps.tile([H, 1], f32, tag="sp", bufs=2)
                nc.tensor.matmul(hp[:], WS0T if t == 0 else WmLT, h1[:],
                                 start=True, stop=True)
                h1n = sb.tile([H, 1], bf16, tag="h1", bufs=2)
                nc.scalar.activation(h1n[:], hp[:], AF.Prelu,
                                     bias=C_sb[:, t:t + 1], alpha=alf[0:H, :])
                h1 = h1n
            h1col = h1

            # ---- GAT row 0, two head-pair blocks, phase-ordered so the
            # Scalar engine runs t0, t1, exp0, exp1 back to back ----
            # phase A: big g_l matmuls (independent of the prologue)
            gl_ps, t_sb, gr0c, e_ps, pexp, ssum = [], [], [], [], [], []
            for b in range(2):
                g = ps.tile([128, N], f32, tag="big", bufs=2)
                nc.tensor.matmul(g[:, 0:512], WlT[b], hT[0:H, 0:512],
                                 start=True, stop=True)
                nc.tensor.matmul(g[:, 512:N_AMB], WlT[b], hT[0:H, 512:N_AMB],
                                 start=True, stop=True)
                gl_ps.append(g)
            # MLP first layer matmul does not depend on the GAT
            y0_ps = ps.tile([H, SHARD], f32, tag="yps", bufs=1)
            nc.tensor.matmul(y0_ps[:], Wd0aT, mlp_sb[:], start=True, stop=True)
            # phase B (after h1): query columns g_r[0] (both blocks into one
            # PSUM tile, one copy), node-0 patch of g_l, then ONE Prelu per
            # block covering all 1024 columns
            gr0_ps = ps.tile([128, 2], f32, tag="sp", bufs=2)
            for b in range(2):
                nc.tensor.matmul(gr0_ps[:, b:b + 1], WrT[:, 128 * b:128 * b + 128],
                                 h1col, start=True, stop=True)
                nc.tensor.matmul(gl_ps[b][:, N_AMB:N], WlT[b], h1col,
                                 start=True, stop=True)
            gr0c2 = sb.tile([128, 2], f32, tag="gr0", bufs=2)
            nc.vector.tensor_copy(gr0c2[:], gr0_ps[:])
            gr0c = [gr0c2[:, 0:1], gr0c2[:, 1:2]]
            # hT's node-0 column: h1 duplicated onto partitions 0-63 / 64-127
            # (feeds only the weighted-mean stt, not the matmuls)
            h1d_ps = ps.tile([128, 1], f32, tag="sp", bufs=2)
            nc.tensor.matmul(h1d_ps[:], dupm, h1[:], start=True, stop=True)
            nc.vector.tensor_copy(hT[:, N_AMB:N], h1d_ps[:])
            for b in range(2):
                t = sb.tile([128, N], bf16, tag="t", bufs=2)
                nc.scalar.activation(t[:], gl_ps[b][:], AF.Prelu,
                                     bias=gr0c[b], alpha=alf[:])
                t_sb.append(t)
            # phase C: e = w_attn . t (replicated x64 on partitions), then
            # Exp with a single accumulate = full softmax numerator + denom
            for b in range(2):
                e = ps.tile([128, N], f32, tag="big", bufs=2)
                nc.tensor.matmul(e[:, 0:512], Wexp, t_sb[b][:, 0:512],
                                 start=True, stop=True)
                nc.tensor.matmul(e[:, 512:N_AMB], Wexp, t_sb[b][:, 512:N_AMB],
                                 start=True, stop=True)
                nc.tensor.matmul(e[:, N_AMB:N], Wexp, t_sb[b][:, N_AMB:N],
                                 start=True, stop=True)
                e_ps.append(e)
            for b in range(2):
                p = sb.tile([128, N], bf16, tag="pexp", bufs=2)
                s = sb.tile([128, 1], f32, tag="s", bufs=8)
                nc.scalar.activation(p[:], e_ps[b][:], AF.Exp, bias=0.0,
                                     accum_out=s[:])
                pexp.append(p)
                ssum.append(s)
            # phase D: weighted mean of h per head, normalize; the head-mean,
            # W_r value projection, AND the MLP's h2 half are all linear, so
            # c0 = Wd0b @ gat_out[0] accumulates directly from the normalized
            # means via host-folded WV_b = 0.25 * Wd0b @ W_r_head_pair_b.T.
            c0_ps = ps.tile([H, 1], f32, tag="h2ps", bufs=1)
            v2, rs = [], []
            for b in range(2):
                scr = sb.tile([128, N], bf16, tag="scr", bufs=2)
                v = sb.tile([128, 1], f32, tag="acc", bufs=8)
                nc.vector.scalar_tensor_tensor(
                    out=scr[:], in0=pexp[b][:], scalar=1.0, in1=hT[:],
                    op0=ALU.mult, op1=ALU.mult, accum_out=v[:])
                v2.append(v)
            for b in range(2):
                r = sb.tile([128, 1], f32, tag="s", bufs=8)
                nc.vector.reciprocal(r[:], ssum[b][:])
                vn2 = sb.tile([128, 1], bf16, tag="acc", bufs=8)
                nc.vector.tensor_tensor(vn2[:], v2[b][:], r[:], op=ALU.mult)
                nc.tensor.matmul(c0_ps[:], WVbT[:, 64 * b:64 * b + 64], vn2[:],
                                 start=(b == 0), stop=(b == 1))

            # ---- final MLP on this core's 128-node shard ----
            c0col = sb.tile([H, 1], f32, tag="c0")
            nc.scalar.activation(c0col[:], c0_ps[:], AF.Identity, bias=bd0c)
            y0 = sb.tile([H, SHARD], bf16, tag="y0")
            nc.scalar.activation(y0[:], y0_ps[:], AF.Prelu, bias=c0col[:],
                                 alpha=alf[0:H, :])
            y1_ps = ps.tile([128, SHARD], f32, tag="yps", bufs=1)
            nc.tensor.matmul(y1_ps[:], Wd1T, y0[:], start=True, stop=True)
            y1 = sb.tile([128, SHARD], bf16, tag="y1")
            nc.scalar.activation(y1[:], y1_ps[:], AF.Prelu, bias=bd1c, alpha=alf[:])
            o_ps = ps.tile([RT, SHARD], f32, tag="sp", bufs=2)
            nc.tensor.matmul(o_ps[:], Wd2T, y1[:], start=True, stop=True)
            # sigmoid(z) = 0.5*tanh(z/2) + 0.5 (Tanh is in the Exp ACT table)
            o_t = sb.tile([RT, SHARD], f32, tag="ot")
            nc.scalar.activation(o_t[:], o_ps[:], AF.Tanh, bias=bd2h, scale=0.5)
            o_sb = sb.tile([RT, SHARD], f32, tag="o")
            nc.vector.tensor_scalar(o_sb[:], o_t[:], 0.5, 0.5,
                                    op0=ALU.mult, op1=ALU.add)
            nc.sync.dma_start(outT_d[:], o_sb[:])

    nc.compile()
    return nc


def _prep_inputs(inputs):
    import ml_dtypes
    f32 = np.float32
    bf16 = ml_dtypes.bfloat16

    def cb(a):
        return np.ascontiguousarray(np.asarray(a, f32).astype(bf16))

    hidden = np.asarray(inputs["hidden"], f32)
    ambiguous = np.asarray(inputs["ambiguous"], f32)
    type_agents = np.asarray(inputs["type_agents"], f32)
    W_self = np.asarray(inputs["W_self"], f32)
    b_self = np.asarray(inputs["b_self"], f32)
    W_merge = np.asarray(inputs["W_merge"], f32)
    b_merge = np.asarray(inputs["b_merge"], f32)
    W_trans = np.asarray(inputs["W_trans"], f32)
    b_trans = np.asarray(inputs["b_trans"], f32)
    W_l = np.asarray(inputs["W_l"], f32)
    W_r = np.asarray(inputs["W_r"], f32)
    w_attn = np.asarray(inputs["w_attn"], f32)
    Wd0 = np.asarray(inputs["Wd0"], f32)
    bd0 = np.asarray(inputs["bd0"], f32)
    Wd1 = np.asarray(inputs["Wd1"], f32)
    bd1 = np.asarray(inputs["bd1"], f32)
    Wd2 = np.asarray(inputs["Wd2"], f32)
    bd2 = np.asarray(inputs["bd2"], f32)

    WmR = W_merge[:, H:]                                   # [64, 64]
    WlT_full = W_l.T                                       # [64, 256]
    WrT_full = W_r.T                                       # [64, 256]
    dup = np.zeros((H, 128), f32)
    dup[np.arange(64), np.arange(64)] = 1.0
    dup[np.arange(64), 64 + np.arange(64)] = 1.0

    ppro = np.zeros((H, 466), f32)
    # WC_t = (WmR @ W_trans[t] / 3).T; column t of C comes from WC_t.T @ tsum_t
    for t in range(RT):
        ppro[:, H * t:H * (t + 1)] = (WmR @ W_trans[t]).T / APT
    WmL = W_merge[:, :H]
    ppro[:, 256:320] = (WmL @ W_self).T
    ppro[:, 320:384] = WmL.T
    ppro[:, 384:396] = type_agents.reshape(RT * APT, H).T
    ppro[:, 396] = hidden[0]
    ppro[:, 397:401] = (b_trans @ WmR.T + b_merge[None, :]).T   # Cbias [64, 4]
    ppro[:, 397] += WmL @ b_self    # folded W_self bias of the first step
    ppro[0, 402:466] = bd0

    pgat = np.zeros((H, 897), f32)
    pgat[:, 0:128] = WlT_full[:, :128]
    pgat[:, 128:256] = WlT_full[:, 128:]
    pgat[:, 256:512] = WrT_full
    pgat[:, 512:640] = dup
    pgat[:, 640:704] = Wd0[:, :H].T
    pgat[:, 704:768] = Wd0[:, H:].T
    pgat[:, 768:896] = Wd1.T
    pgat[:, 896] = bd0

    pb128 = np.zeros((128, 262), f32)
    Wexp = np.zeros((128, 128), f32)
    for hh in range(2):
        Wexp[hh * 64:(hh + 1) * 64, hh * 64:(hh + 1) * 64] = w_attn[:, None]
    pb128[:, 0:128] = Wexp
    # WV_b.T where WV_b = 0.25 * Wd0[:, 64:] @ [Wr_head_even | Wr_head_odd].T
    # stacked so that rows 0-63 pair with v_even, 64-127 with v_odd
    Wd0b = Wd0[:, H:]
    for b in range(2):
        WrT2q = np.zeros((128, 64), f32)
        WrT2q[0:64, :] = 0.25 * WrT_full[:, 128 * b:128 * b + 64]
        WrT2q[64:128, :] = 0.25 * WrT_full[:, 128 * b + 64:128 * b + 128]
        pb128[:, 128 + 64 * b:128 + 64 * b + 64] = WrT2q @ Wd0b.T
    pb128[:, 256:260] = Wd2.T
    pb128[:, 260] = bd1
    pb128[0:RT, 261] = 0.5 * bd2

    shared = {
        "ppro": cb(ppro),
        "pgat": cb(pgat),
        "pb128": cb(pb128),
        "amb": cb(ambiguous.T),
    }
    amb_pad = np.zeros((H, NCORES * SHARD), f32)
    amb_pad[:, :N_AMB] = ambiguous.T
    in_maps = []
    for cidx in range(NCORES):
        m = dict(shared)
        m["mlp_cols"] = cb(amb_pad[:, cidx * SHARD:(cidx + 1) * SHARD])
        in_maps.append(m)
    return in_maps


def kernel(**inputs) -> np.ndarray:
    global _compiled
    if _compiled is None:
        _compiled = _build()
    nc = _compiled
    from concourse import bass_utils

    in_maps = _prep_inputs(inputs)
    res = bass_utils.run_bass_kernel_spmd(nc, in_maps, core_ids=list(range(NCORES)))
    out = np.empty((N_AMB, RT), np.float32)
    for cidx in range(NCORES):
        lo = cidx * SHARD
        hi = min(lo + SHARD, N_AMB)
        out[lo:hi, :] = res.results[cidx]["outT"][:, :hi - lo].T
    return out


# revision 57
# speedup vs baseline: 1.0501x; 1.0501x over previous
"""Trainium2 Bass kernel for nn_MlroleNode_64716567216639 (GAT message passing).

Math note: the reference model computes a dense NxN GATv2 attention but only
row 0 of the output (gat_out[0]) feeds the final MLP, so this kernel computes
just that row: e[j,h] = leaky(g_l[j] + g_r[0]) . w_attn, softmax over the 1024
source nodes.  Because g_r is linear in h, the weighted value sum collapses to
sum_j a[j] g_r[j] = W_r @ (sum_j a[j] h[j]), so only per-head weighted means of
h are accumulated (no big g_r tensor).  The final 3-layer type-define MLP runs
on the 1023 ambiguous nodes, sharded 128 nodes per core.

Layout: features on partitions, nodes on the free axis.  h is stored as
hT[128, 1024] with the 64 features duplicated on partitions 0-63 and 64-127
(one elementwise multiply covers both heads of a head-pair block), columns
0..1022 = ambiguous nodes (DMA'd), column 1023 = node 0 (= h1, computed by the
on-device prologue and patched in afterwards so the big GAT matmuls do not
wait on the serial prologue chain).  bf16 everywhere except softmax sums and
the output (tolerance is 2e-2; bf16 keeps rel err ~2e-3).  The role-merge
prologue uses host-folded WC_t = W_merge[:,64:] @ W_trans[t] / 3 so the
routing MLPs collapse into 4 small matmuls.  sigmoid(z) = 0.5*tanh(z/2)+0.5
avoids the slow DVE reciprocal.  Block 0's weighted mean runs on the Vector
engine, block 1's on GpSimd, so the two blocks' softmax tails overlap.
"""
import numpy as np

H = 64
N_AMB = 1023
N = 1024
HEADS = 4
HID = 64
RT = 4
APT = 3
SLOPE = 0.2
NCORES = 8
SHARD = 128  # MLP nodes per core (8*128 = 1024 = N_AMB padded by 1)

_compiled = None


def _build():
    import concourse.tile as tile
    from concourse import bacc, mybir

    f32 = mybir.dt.float32
    bf16 = mybir.dt.bfloat16
    AF = mybir.ActivationFunctionType
    ALU = mybir.AluOpType
    AX = mybir.AxisListType

    nc = bacc.Bacc("TRN2", target_bir_lowering=False, debug=False,
                   enable_asserts=False, num_devices=NCORES)

    # ---- DRAM inputs (packed on host to minimize DMA count) ----
    # ppro bf16 [64, 466]: WCt(256) WselfT(64) WmLT(64) ta(12) hidc(1)
    #   Cbias(4) bsc(1) bd0row(64, on partition 0 only)
    ppro_d = nc.dram_tensor("ppro", [H, 466], bf16, kind="ExternalInput").ap()
    # pgat bf16 [64, 897]: WlT0(128) WlT1(128) WrT(256) dup(128) Wd0aT(64)
    #   Wd0bT(64) Wd1T(128) bd0(1)
    pgat_d = nc.dram_tensor("pgat", [H, 897], bf16, kind="ExternalInput").ap()
    # pb128 bf16 [128, 262]: Wexp(128) WrT2q(128, pre-scaled 0.25) Wd2T(4)
    #   bd1(1) bd2h(1, = bd2/2 for the tanh-based sigmoid)
    pb128_d = nc.dram_tensor("pb128", [128, 262], bf16, kind="ExternalInput").ap()
    amb_d = nc.dram_tensor("amb", [H, N_AMB], bf16, kind="ExternalInput").ap()
    mlp_d = nc.dram_tensor("mlp_cols", [H, SHARD], bf16, kind="ExternalInput").ap()
    outT_d = nc.dram_tensor("outT", [RT, SHARD], f32, kind="ExternalOutput").ap()

    with tile.TileContext(nc) as tc:
        with tc.tile_pool(name="wp", bufs=1) as wp, \
             tc.tile_pool(name="sb", bufs=1) as sb, \
             tc.tile_pool(name="ps", bufs=1, space="PSUM") as ps:

            # ---- ACT table preload: first scalar-engine instruction so the
            # Exp/Prelu/Tanh table load runs before the scalar-queue DMAs.
            warm = wp.tile([1, 4], f32, tag="warm")
            nc.vector.memset(warm[:], 0.0)
            warm_act = wp.tile([1, 4], f32, tag="warmact")
            nc.scalar.activation(warm_act[0:1, 0:1], warm[0:1, 0:1], AF.Exp)
            alf = wp.tile([128, 1], f32, tag="alf")
            nc.vector.memset(alf[:], SLOPE)

            # ---- input DMAs, split across the two HWDGE queues ----
            ppro = wp.tile([H, 466], bf16, tag="ppro")
            pgat = wp.tile([H, 897], bf16, tag="pgat")
            pb128 = wp.tile([128, 262], bf16, tag="pb128")
            hT = wp.tile([128, N], bf16, tag="hT")
            mlp_sb = wp.tile([H, SHARD], bf16, tag="mlp")
            nc.sync.dma_start(ppro[:], ppro_d[:])
            nc.sync.dma_start(pgat[:], pgat_d[:])
            nc.scalar.dma_start(hT[0:H, 0:N_AMB], amb_d[:])
            nc.scalar.dma_start(mlp_sb[:], mlp_d[:])
            nc.scalar.dma_start(pb128[:], pb128_d[:])
            # duplicate the feature rows onto partitions 64-127 on-chip
            # (SBUF->SBUF DMA; halves the HBM read traffic per core)
            nc.sync.dma_start(hT[H:128, 0:N_AMB], hT[0:H, 0:N_AMB])

            # views into the packs
            WCt = ppro[:, 0:256]
            WS0T = ppro[:, 256:320]
            WmLT = ppro[:, 320:384]
            ta_sb = ppro[:, 384:396]
            hidc = ppro[:, 396:397]
            Cbias = ppro[:, 397:401]
            bsc = ppro[:, 401:402]
            bd0r = ppro[0:1, 402:466]
            WlT = [pgat[:, 0:128], pgat[:, 128:256]]
            WrT = pgat[:, 256:512]
            dupm = pgat[:, 512:640]
            Wd0aT = pgat[:, 640:704]
            Wd0bT = pgat[:, 704:768]
            Wd1T = pgat[:, 768:896]
            bd0c = pgat[:, 896:897]
            Wexp = pb128[:, 0:128]
            WVbT = pb128[:, 128:256]
            Wd2T = pb128[:, 256:260]
            bd1c = pb128[:, 260:261]
            bd2h = pb128[0:RT, 261:262]

            # ---- prologue: role routing (host-folded WC) + merge chain ----
            tsum = sb.tile([H, RT], bf16, tag="tsum")
            with nc.allow_low_precision(reason="sum of 3 bf16 agent vectors"):
                nc.vector.reduce_sum(tsum[:],
                                     ta_sb.rearrange("p (t a) -> p t a", a=APT),
                                     axis=AX.X)
            C_ps = ps.tile([H, RT], f32, tag="sp", bufs=2)
            for t in range(RT):
                nc.tensor.matmul(C_ps[:, t:t + 1], WCt[:, H * t:H * (t + 1)],
                                 tsum[:, t:t + 1], start=True, stop=True)
            C_sb = sb.tile([H, RT], f32, tag="C")
            nc.vector.tensor_tensor(C_sb[:], C_ps[:], Cbias, op=ALU.add)

            # step 0 consumes `hidden` directly: WS0 = WmL @ W_self and
            # WmL @ b_self are folded on the host (into WS0T / Cbias col 0)
            h1 = hidc
            for t in range(RT):
                hp = ps.tile([H, 1], f32, tag="sp", bufs=2)
                nc.tensor.matmul(hp[:], WS0T if t == 0 else WmLT, h1[:],
                                 start=True, stop=True)
                h1n = sb.tile([H, 1], bf16, tag="h1", bufs=2)
                nc.scalar.activation(h1n[:], hp[:], AF.Prelu,
                                     bias=C_sb[:, t:t + 1], alpha=alf[0:H, :])
                h1 = h1n
            h1col = h1

            # ---- GAT row 0, two head-pair blocks, phase-ordered so the
            # Scalar engine runs t0, t1, exp0, exp1 back to back ----
            # phase A: big g_l matmuls (independent of the prologue)
            gl_ps, t_sb, gr0c, e_ps, pexp, ssum = [], [], [], [], [], []
            for b in range(2):
                g = ps.tile([128, N], f32, tag="big", bufs=2)
                nc.tensor.matmul(g[:, 0:512], WlT[b], hT[0:H, 0:512],
                                 start=True, stop=True)
                nc.tensor.matmul(g[:, 512:N_AMB], WlT[b], hT[0:H, 512:N_AMB],
                                 start=True, stop=True)
                gl_ps.append(g)
            # MLP first layer matmul does not depend on the GAT
            y0_ps = ps.tile([H, SHARD], f32, tag="yps", bufs=1)
            nc.tensor.matmul(y0_ps[:], Wd0aT, mlp_sb[:], start=True, stop=True)
            # phase B (after h1): query column g_r[0], node-0 patch of g_l,
            # then ONE Prelu per block covering all 1024 columns
            for b in range(2):
                gr0_ps = ps.tile([128, 1], f32, tag="sp", bufs=2)
                nc.tensor.matmul(gr0_ps[:], WrT[:, 128 * b:128 * b + 128], h1col,
                                 start=True, stop=True)
                c = sb.tile([128, 1], f32, tag="gr0", bufs=2)
                nc.vector.tensor_copy(c[:], gr0_ps[:])
                gr0c.append(c)
                nc.tensor.matmul(gl_ps[b][:, N_AMB:N], WlT[b], h1col,
                                 start=True, stop=True)
            # hT's node-0 column: h1 duplicated onto partitions 0-63 / 64-127
            # (feeds only the weighted-mean stt, not the matmuls)
            h1d_ps = ps.tile([128, 1], f32, tag="sp", bufs=2)
            nc.tensor.matmul(h1d_ps[:], dupm, h1[:], start=True, stop=True)
            nc.vector.tensor_copy(hT[:, N_AMB:N], h1d_ps[:])
            for b in range(2):
                t = sb.tile([128, N], bf16, tag="t", bufs=2)
                nc.scalar.activation(t[:], gl_ps[b][:], AF.Prelu,
                                     bias=gr0c[b][:], alpha=alf[:])
                t_sb.append(t)
            # phase C: e = w_attn . t (replicated x64 on partitions), then
            # Exp with a single accumulate = full softmax numerator + denom
            for b in range(2):
                e = ps.tile([128, N], f32, tag="big", bufs=2)
                nc.tensor.matmul(e[:, 0:512], Wexp, t_sb[b][:, 0:512],
                                 start=True, stop=True)
                nc.tensor.matmul(e[:, 512:N_AMB], Wexp, t_sb[b][:, 512:N_AMB],
                                 start=True, stop=True)
                nc.tensor.matmul(e[:, N_AMB:N], Wexp, t_sb[b][:, N_AMB:N],
                                 start=True, stop=True)
                e_ps.append(e)
            for b in range(2):
                p = sb.tile([128, N], bf16, tag="pexp", bufs=2)
                s = sb.tile([128, 1], f32, tag="s", bufs=8)
                nc.scalar.activation(p[:], e_ps[b][:], AF.Exp, bias=0.0,
                                     accum_out=s[:])
                pexp.append(p)
                ssum.append(s)
            # phase D: weighted mean of h per head, normalize; the head-mean,
            # W_r value projection, AND the MLP's h2 half are all linear, so
            # c0 = Wd0b @ gat_out[0] accumulates directly from the normalized
            # means via host-folded WV_b = 0.25 * Wd0b @ W_r_head_pair_b.T.
            c0_ps = ps.tile([H, 1], f32, tag="h2ps", bufs=1)
            v2, rs = [], []
            for b in range(2):
                scr = sb.tile([128, N], bf16, tag="scr", bufs=2)
                v = sb.tile([128, 1], f32, tag="acc", bufs=8)
                nc.vector.scalar_tensor_tensor(
                    out=scr[:], in0=pexp[b][:], scalar=1.0, in1=hT[:],
                    op0=ALU.mult, op1=ALU.mult, accum_out=v[:])
                v2.append(v)
            for b in range(2):
                r = sb.tile([128, 1], f32, tag="s", bufs=8)
                nc.vector.reciprocal(r[:], ssum[b][:])
                vn2 = sb.tile([128, 1], bf16, tag="acc", bufs=8)
                nc.vector.tensor_tensor(vn2[:], v2[b][:], r[:], op=ALU.mult)
                nc.tensor.matmul(c0_ps[:], WVbT[:, 64 * b:64 * b + 64], vn2[:],
                                 start=(b == 0), stop=(b == 1))

            # ---- final MLP on this core's 128-node shard ----
            c0col = sb.tile([H, 1], f32, tag="c0")
            nc.scalar.activation(c0col[:], c0_ps[:], AF.Identity, bias=bd0c)
            y0 = sb.tile([H, SHARD], bf16, tag="y0")
            nc.scalar.activation(y0[:], y0_ps[:], AF.Prelu, bias=c0col[:],
                                 alpha=alf[0:H, :])
            y1_ps = ps.tile([128, SHARD], f32, tag="yps", bufs=1)
            nc.tensor.matmul(y1_ps[:], Wd1T, y0[:], start=True, stop=True)
            y1 = sb.tile([128, SHARD], bf16, tag="y1")
            nc.scalar.activation(y1[:], y1_ps[:], AF.Prelu, bias=bd1c, alpha=alf[:])
            o_ps = ps.tile([RT, SHARD], f32, tag="sp", bufs=2)
            nc.tensor.matmul(o_ps[:], Wd2T, y1[:], start=True, stop=True)
            # sigmoid(z) = 0.5*tanh(z/2) + 0.5 (Tanh is in the Exp ACT table)
            o_t = sb.tile([RT, SHARD], f32, tag="ot")
            nc.scalar.activation(o_t[:], o_ps[:], AF.Tanh, bias=bd2h, scale=0.5)
            o_sb = sb.tile([RT, SHARD], f32, tag="o")
            nc.vector.tensor_scalar(o_sb[:], o_t[:], 0.5, 0.5,
                                    op0=ALU.mult, op1=ALU.add)
            nc.sync.dma_start(outT_d[:], o_sb[:])

    nc.compile()
    return nc


def _prep_inputs(inputs):
    import ml_dtypes
    f32 = np.float32
    bf16 = ml_dtypes.bfloat16

    def cb(a):
        return np.ascontiguousarray(np.asarray(a, f32).astype(bf16))

    hidden = np.asarray(inputs["hidden"], f32)
    ambiguous = np.asarray(inputs["ambiguous"], f32)
    type_agents = np.asarray(inputs["type_agents"], f32)
    W_self = np.asarray(inputs["W_self"], f32)
    b_self = np.asarray(inputs["b_self"], f32)
    W_merge = np.asarray(inputs["W_merge"], f32)
    b_merge = np.asarray(inputs["b_merge"], f32)
    W_trans = np.asarray(inputs["W_trans"], f32)
    b_trans = np.asarray(inputs["b_trans"], f32)
    W_l = np.asarray(inputs["W_l"], f32)
    W_r = np.asarray(inputs["W_r"], f32)
    w_attn = np.asarray(inputs["w_attn"], f32)
    Wd0 = np.asarray(inputs["Wd0"], f32)
    bd0 = np.asarray(inputs["bd0"], f32)
    Wd1 = np.asarray(inputs["Wd1"], f32)
    bd1 = np.asarray(inputs["bd1"], f32)
    Wd2 = np.asarray(inputs["Wd2"], f32)
    bd2 = np.asarray(inputs["bd2"], f32)

    WmR = W_merge[:, H:]                                   # [64, 64]
    WlT_full = W_l.T                                       # [64, 256]
    WrT_full = W_r.T                                       # [64, 256]
    dup = np.zeros((H, 128), f32)
    dup[np.arange(64), np.arange(64)] = 1.0
    dup[np.arange(64), 64 + np.arange(64)] = 1.0

    ppro = np.zeros((H, 466), f32)
    # WC_t = (WmR @ W_trans[t] / 3).T; column t of C comes from WC_t.T @ tsum_t
    for t in range(RT):
        ppro[:, H * t:H * (t + 1)] = (WmR @ W_trans[t]).T / APT
    WmL = W_merge[:, :H]
    ppro[:, 256:320] = (WmL @ W_self).T
    ppro[:, 320:384] = WmL.T
    ppro[:, 384:396] = type_agents.reshape(RT * APT, H).T
    ppro[:, 396] = hidden[0]
    ppro[:, 397:401] = (b_trans @ WmR.T + b_merge[None, :]).T   # Cbias [64, 4]
    ppro[:, 397] += WmL @ b_self    # folded W_self bias of the first step
    ppro[0, 402:466] = bd0

    pgat = np.zeros((H, 897), f32)
    pgat[:, 0:128] = WlT_full[:, :128]
    pgat[:, 128:256] = WlT_full[:, 128:]
    pgat[:, 256:512] = WrT_full
    pgat[:, 512:640] = dup
    pgat[:, 640:704] = Wd0[:, :H].T
    pgat[:, 704:768] = Wd0[:, H:].T
    pgat[:, 768:896] = Wd1.T
    pgat[:, 896] = bd0

    pb128 = np.zeros((128, 262), f32)
    Wexp = np.zeros((128, 128), f32)
    for hh in range(2):
        Wexp[hh * 64:(hh + 1) * 64, hh * 64:(hh + 1) * 64] = w_attn[:, None]
    pb128[:, 0:128] = Wexp
    # WV_b.T where WV_b = 0.25 * Wd0[:, 64:] @ [Wr_head_even | Wr_head_odd].T
    # stacked so that rows 0-63 pair with v_even, 64-127 with v_odd
    Wd0b = Wd0[:, H:]
    for b in range(2):
        WrT2q = np.zeros((128, 64), f32)
        WrT2q[0:64, :] = 0.25 * WrT_full[:, 128 * b:128 * b + 64]
        WrT2q[64:128, :] = 0.25 * WrT_full[:, 128 * b + 64:128 * b + 128]
        pb128[:, 128 + 64 * b:128 + 64 * b + 64] = WrT2q @ Wd0b.T
    pb128[:, 256:260] = Wd2.T
    pb128[:, 260] = bd1
    pb128[0:RT, 261] = 0.5 * bd2

    shared = {
        "ppro": cb(ppro),
        "pgat": cb(pgat),
        "pb128": cb(pb128),
        "amb": cb(ambiguous.T),
    }
    amb_pad = np.zeros((H, NCORES * SHARD), f32)
    amb_pad[:, :N_AMB] = ambiguous.T
    in_maps = []
    for cidx in range(NCORES):
        m = dict(shared)
        m["mlp_cols"] = cb(amb_pad[:, cidx * SHARD:(cidx + 1) * SHARD])
        in_maps.append(m)
    return in_maps


def kernel(**inputs) -> np.ndarray:
    global _compiled
    if _compiled is None:
        _compiled = _build()
    nc = _compiled
    from concourse import bass_utils

    in_maps = _prep_inputs(inputs)
    res = bass_utils.run_bass_kernel_spmd(nc, in_maps, core_ids=list(range(NCORES)))
    out = np.empty((N_AMB, RT), np.float32)
    for cidx in range(NCORES):
        lo = cidx * SHARD
        hi = min(lo + SHARD, N_AMB)
        out[lo:hi, :] = res.results[cidx]["outT"][:, :hi - lo].T
    return out
